# revision 1
# baseline (speedup 1.0000x reference)
"""LogGaborConv2d on 8 TRN2 NeuronCores.

Strategy: data-parallel over batch (8 images -> 8 cores). Per core:
- Gabor weights [O=128, I=64, 3, 3] computed on device from the params.
- 3x3 conv as 9 accumulating matmuls (K=64 input channels) over a
  column-padded flat image stream (width 258), windows of 512 pixels
  into PSUM banks.
- The 128 PE rows are split into two row-groups: partitions 0:64
  process the top half of the image, partitions 64:128 the bottom half,
  as concurrent K=64 matmuls (tile_position row groups), doubling PE
  throughput vs a single K=64 stream.
- fp32r matmul dtype: full-rate (1 cycle/row) with ~1e-4 relative error.

Host side only pads/shards inputs and de-pads/gathers outputs.
"""
import math

import numpy as np

import concourse.bacc as bacc
import concourse.bass as bass  # noqa: F401
import concourse.mybir as mybir
import concourse.tile as tile
from concourse.bass_utils import run_bass_kernel_spmd

F32 = mybir.dt.float32
F32R = mybir.dt.float32r
AF = mybir.ActivationFunctionType
OP = mybir.AluOpType

# problem constants
NB, C, H, W = 8, 64, 256, 256
O = 128
WP = W + 2            # padded row width
SL = (H + 2) * WP     # padded input stream length (incl. top/bottom pad rows)
OL = H * WP           # padded output stream length
NWIN = OL // 512      # 129 windows of 512
GUARD = 4             # leading guard zeros in the host-side stream
TLEN = 512 * 8 + 524  # input tile covers 8 windows + halo
TLEN_MINI = 512 + 524
XLEN = 512 * 128 + TLEN_MINI + GUARD  # 66572+4 -> round up
XLEN = (XLEN + 15) // 16 * 16
# grid values from reference: linspace(-1, 2, 3) both axes
_GRID = (-1.0, 0.5, 2.0)
DELTA = 0.001
NW_A = 64             # windows handled by partitions 0:64
# windows NW_A..128 handled by partitions 64:128


def _taps():
    """(tap_index, ky, kx, delta, r, exp_scale) for the 9 taps."""
    out = []
    for ky in range(3):
        for kx in range(3):
            t = 3 * ky + kx
            delta = ky * WP + (kx - 1)
            r2 = _GRID[kx] ** 2 + _GRID[ky] ** 2 + DELTA
            r = math.sqrt(r2)
            esc = -(math.log(r) ** 2) / 4.0
            out.append((t, ky, kx, delta, r, esc))
    return out


def build_kernel():
    nc = bacc.Bacc("TRN2", target_bir_lowering=False)
    x = nc.dram_tensor("x", [C, XLEN], F32R, kind="ExternalInput")
    params = nc.dram_tensor("params", [C, 512], F32, kind="ExternalInput")
    y = nc.dram_tensor("y", [O, OL], F32, kind="ExternalOutput")

    taps = _taps()

    with tile.TileContext(nc) as tc:
        with (
            tc.tile_pool(name="wg", bufs=1) as wg,
            tc.tile_pool(name="xin", bufs=2) as xin,
            tc.tile_pool(name="outp", bufs=3) as outp,
            tc.tile_pool(name="ps", bufs=2, space="PSUM") as ps,
        ):
            # ---------------- weight generation ----------------
            par = wg.tile([C, 512], F32)
            nc.sync.dma_start(par[:], params[:])
            th = par[:, 0:128]
            sg = par[:, 128:256]
            fr = par[:, 256:384]
            pss = par[:, 384:512]

            lnsg = wg.tile([C, 128], F32)
            nc.scalar.activation(lnsg[:], sg, AF.Ln)
            lsq = wg.tile([C, 128], F32)
            nc.vector.tensor_mul(lsq[:], lnsg[:], lnsg[:])
            il2 = wg.tile([C, 128], F32)
            nc.vector.reciprocal(il2[:], lsq[:])
            sg2 = wg.tile([C, 128], F32)
            nc.vector.tensor_mul(sg2[:], sg, sg)
            sinv = wg.tile([C, 128], F32)
            nc.vector.reciprocal(sinv[:], sg2[:])
            thm1 = wg.tile([C, 128], F32)
            nc.vector.tensor_scalar(thm1[:], th, 1.0, None, OP.subtract)
            a2 = wg.tile([C, 128], F32)
            nc.vector.tensor_mul(a2[:], thm1[:], thm1[:])
            asv = wg.tile([C, 128], F32)
            nc.vector.tensor_mul(asv[:], a2[:], sinv[:])
            e2 = wg.tile([C, 128], F32)
            nc.scalar.activation(e2[:], asv[:], AF.Exp, scale=-0.5)
            m1 = wg.tile([C, 128], F32)
            nc.vector.scalar_tensor_tensor(
                m1[:], e2[:], 1.0 / (2.0 * math.pi), sinv[:], OP.mult, OP.mult
            )

            argb = wg.tile([C, 1152], F32)
            eb = wg.tile([C, 1152], F32)
            for t, ky, kx, delta, r, esc in taps:
                nc.vector.scalar_tensor_tensor(
                    argb[:, 128 * t : 128 * t + 128], fr, float(r), pss,
                    OP.mult, OP.add,
                )
                nc.vector.tensor_scalar(
                    eb[:, 128 * t : 128 * t + 128], il2[:], float(esc), None,
                    OP.mult,
                )
            # cos(v) = sin(pi/2 - v), folded into [-pi, pi]
            wv = wg.tile([C, 1152], F32)
            nc.vector.tensor_scalar(
                wv[:], argb[:], -1.0, math.pi / 2.0, OP.mult, OP.add
            )
            msk = wg.tile([C, 1152], F32)
            nc.vector.tensor_single_scalar(msk[:], wv[:], -math.pi, OP.is_lt)
            wv2 = wg.tile([C, 1152], F32)
            nc.vector.scalar_tensor_tensor(
                wv2[:], msk[:], 2.0 * math.pi, wv[:], OP.mult, OP.add
            )
            cosb = wg.tile([C, 1152], F32)
            nc.scalar.activation(cosb[:], wv2[:], AF.Sin)
            e1b = wg.tile([C, 1152], F32)
            nc.scalar.activation(e1b[:], eb[:], AF.Exp)
            ecb = wg.tile([C, 1152], F32)
            nc.vector.tensor_mul(ecb[:], e1b[:], cosb[:])
            wt = wg.tile([O, 1152], F32R)
            for t, ky, kx, delta, r, esc in taps:
                nc.vector.tensor_mul(
                    wt[0:C, 128 * t : 128 * t + 128],
                    ecb[:, 128 * t : 128 * t + 128],
                    m1[:],
                )
            # duplicate weights into partitions 64:128 for the B row-group
            nc.sync.dma_start(wt[C : 2 * C, :], wt[0:C, :])

            # ---------------- convolution ----------------
            def emit_group(wa0, na, wb0, nb, xt, w0a, w0b):
                pa = [
                    ps.tile([O, 512], F32, tag=f"a{j}", name=f"pa{j}")
                    for j in range(na)
                ]
                pb = [
                    ps.tile([O, 512], F32, tag=f"b{j}", name=f"pb{j}")
                    for j in range(nb)
                ]
                ntap = len(taps)
                for t, ky, kx, delta, r, esc in taps:
                    lhs_a = wt[0:C, 128 * t : 128 * t + 128]
                    lhs_b = wt[C : 2 * C, 128 * t : 128 * t + 128]
                    first = t == 0
                    last = t == ntap - 1
                    for j in range(max(na, nb)):
                        if j < na:
                            o = 512 * (wa0 + j - w0a) + delta + GUARD
                            nc.tensor.matmul(
                                pa[j][:], lhs_a, xt[0:C, o : o + 512],
                                start=first, stop=last,
                            )
                        if j < nb:
                            o = 512 * (wb0 + j - w0b) + delta + GUARD
                            nc.tensor.matmul(
                                pb[j][:], lhs_b, xt[C : 2 * C, o : o + 512],
                                start=first, stop=last,
                            )
                ot = outp.tile([O, 512 * (na + nb)], F32, tag="ot", name="ot")
                for j in range(na):
                    eng = nc.scalar if j % 2 == 0 else nc.vector
                    if eng is nc.scalar:
                        nc.scalar.copy(ot[:, 512 * j : 512 * j + 512], pa[j][:])
                    else:
                        nc.vector.tensor_copy(
                            ot[:, 512 * j : 512 * j + 512], pa[j][:]
                        )
                for j in range(nb):
                    c0 = 512 * (na + j)
                    if j % 2 == 1:
                        nc.scalar.copy(ot[:, c0 : c0 + 512], pb[j][:])
                    else:
                        nc.vector.tensor_copy(ot[:, c0 : c0 + 512], pb[j][:])
                if na:
                    nc.sync.dma_start(
                        y[:, 512 * wa0 : 512 * (wa0 + na)], ot[:, 0 : 512 * na]
                    )
                if nb:
                    nc.sync.dma_start(
                        y[:, 512 * wb0 : 512 * (wb0 + nb)],
                        ot[:, 512 * na : 512 * (na + nb)],
                    )

            for tblk in range(8):
                w0a = 8 * tblk
                w0b = NW_A + 8 * tblk
                xt = xin.tile([2 * C, TLEN], F32R, tag="xt", name="xt")
                nc.sync.dma_start(
                    xt[0:C, :], x[:, 512 * w0a : 512 * w0a + TLEN]
                )
                nc.sync.dma_start(
                    xt[C : 2 * C, :], x[:, 512 * w0b : 512 * w0b + TLEN]
                )
                for sub in range(4):
                    emit_group(
                        w0a + 2 * sub, 2, w0b + 2 * sub, 2, xt, w0a, w0b
                    )
            # final window 128 on the B row-group
            xtm = xin.tile([2 * C, TLEN], F32R, tag="xt", name="xtm")
            nc.sync.dma_start(
                xtm[C : 2 * C, 0:TLEN_MINI],
                x[:, 512 * 128 : 512 * 128 + TLEN_MINI],
            )
            emit_group(0, 0, 128, 1, xtm, 0, 128)

    nc.compile()
    return nc


_NC_CACHE = None


def _get_nc():
    global _NC_CACHE
    if _NC_CACHE is None:
        _NC_CACHE = build_kernel()
    return _NC_CACHE


def kernel(input_tensor, freq, theta, sigma, psi, f0, theta0, xg, yg):
    x = np.ascontiguousarray(np.asarray(input_tensor, dtype=np.float32))
    params = np.ascontiguousarray(
        np.concatenate(
            [
                np.asarray(theta, np.float32).T,
                np.asarray(sigma, np.float32).T,
                np.asarray(freq, np.float32).T,
                np.asarray(psi, np.float32).T,
            ],
            axis=1,
        )
    )
    nc = _get_nc()
    in_maps = []
    for c in range(NB):
        xp = np.zeros((C, XLEN), np.float32)
        view = xp[:, GUARD : GUARD + SL].reshape(C, H + 2, WP)
        view[:, 1 : H + 1, 1 : W + 1] = x[c]
        in_maps.append({"x": xp, "params": params})
    res = run_bass_kernel_spmd(nc, in_maps, core_ids=list(range(NB)))
    out = np.empty((NB, O, H, W), np.float32)
    for c in range(NB):
        out[c] = res.results[c]["y"].reshape(O, H, WP)[:, :, 1 : W + 1]
    return out



# revision 3
# speedup vs baseline: 1.6737x; 1.6737x over previous
"""LogGaborConv2d on 8 TRN2 NeuronCores.

Strategy: data-parallel over batch (8 images -> 8 cores). Key ideas:

- The log-Gabor weights depend on the 3x3 grid only through
  r^2 = x^2 + y^2, so the kernel is symmetric across its diagonal:
  w[0,1]==w[1,0], w[0,2]==w[2,0], w[1,2]==w[2,1] -> only 6 unique taps.
  With on-chip pair-sum streams S1[q] = x[q] + x[q+257] and
  S2[q] = x[q] + x[q+514], the 9 matmuls per output window collapse
  to 6 (1.5x less PE work).
- bf16 weights (host-computed from the tiny Gabor params) + bf16 input
  stream + bf16 output: halves HBM traffic on both sides (the per-core
  HBM limit of ~358 GB/s would otherwise bound the kernel) and enables
  fast weight load. Matmuls accumulate in fp32 PSUM; measured rel err
  ~4e-3 vs the fp32 reference, well under the 2e-2 gate.
- Two PE row groups: partitions 0:64 process windows 0..63 of the
  padded output stream, partitions 64:128 windows 64..128, as
  concurrent K=64 matmuls.
- PSUM tiles span 2 banks (2 windows) so each PSUM->SBUF copy moves
  1024 columns, halving per-op overhead on the scalar/vector engines.

Host side computes the 6 unique [64,128] weight blocks in numpy,
pads/shards inputs, and de-pads/gathers outputs.
"""
import math

import ml_dtypes
import numpy as np

import concourse.bacc as bacc
import concourse.bass as bass  # noqa: F401
import concourse.mybir as mybir
import concourse.tile as tile
from concourse.bass_utils import run_bass_kernel_spmd

F32 = mybir.dt.float32
BF16 = mybir.dt.bfloat16
BF16_NP = ml_dtypes.bfloat16

# problem constants
NB, C, H, W = 8, 64, 256, 256
O = 128
WP = W + 2            # padded row width
SL = (H + 2) * WP     # padded input stream length (incl. top/bottom pad rows)
OL = H * WP           # padded output stream length
NWIN = OL // 512      # 129 windows of 512
GUARD = 4             # leading guard zeros in the host-side stream
XLEN = 66576          # GUARD + SL rounded up; covers the mini tile too
TLEN = 512 * 8 + 524  # input tile: 8 windows + halo
TLEN_MINI = 512 + 524
L1 = TLEN - 257       # S1 pair-sum stream length per tile
L2 = TLEN - 514       # S2 pair-sum stream length per tile
NW_A = 64             # windows handled by partitions 0:64

# 6 unique taps after diagonal merge. Each entry:
#   (src, off) where src: 0=xt, 1=s1, 2=s2 and off is the in-tile
#   column offset for window j=0 (window j adds 512*j).
# Accumulation order puts the xt-only taps first so the pair-sum
# streams get maximum scheduling slack.
# tap (ky,kx) offset = ky*WP + (kx-1) + GUARD into the raw stream.
TAPS = [
    ((0, 0), 0, GUARD - 1),        # w00: xt at -1
    ((1, 1), 0, GUARD + WP),       # w11: xt at 258
    ((2, 2), 0, GUARD + 2 * WP + 1),  # w22: xt at 517
    ((0, 1), 1, GUARD),            # w01 (=w10): S1 at 0
    ((1, 2), 1, GUARD + WP + 1),   # w12 (=w21): S1 at 259
    ((0, 2), 2, GUARD + 1),        # w02 (=w20): S2 at 1
]


def build_kernel():
    nc = bacc.Bacc("TRN2", target_bir_lowering=False)
    x = nc.dram_tensor("x", [C, XLEN], BF16, kind="ExternalInput")
    wt_in = nc.dram_tensor("wt", [O, 6 * 128], BF16, kind="ExternalInput")
    y = nc.dram_tensor("y", [O, OL], BF16, kind="ExternalOutput")

    with tile.TileContext(nc) as tc:
        with (
            tc.tile_pool(name="wg", bufs=1) as wg,
            tc.tile_pool(name="xin", bufs=3) as xin,
            tc.tile_pool(name="s1p", bufs=2) as s1p,
            tc.tile_pool(name="s2p", bufs=2) as s2p,
            tc.tile_pool(name="outp", bufs=3) as outp,
            tc.tile_pool(name="ps", bufs=2, space="PSUM") as ps,
        ):
            wt = wg.tile([O, 6 * 128], BF16)
            nc.sync.dma_start(wt[:], wt_in[:])

            def make_tiles(w0a, w0b, tlen, l1, l2, b_only=False):
                xt = xin.tile([O, TLEN], BF16, tag="xt", name="xt")
                if not b_only:
                    nc.sync.dma_start(
                        xt[0:C, 0:tlen], x[:, 512 * w0a : 512 * w0a + tlen]
                    )
                nc.sync.dma_start(
                    xt[C:O, 0:tlen], x[:, 512 * w0b : 512 * w0b + tlen]
                )
                s1 = s1p.tile([O, L1], BF16, tag="s1", name="s1")
                s2 = s2p.tile([O, L2], BF16, tag="s2", name="s2")
                p0 = C if b_only else 0
                nc.vector.tensor_add(
                    s1[p0:O, 0:l1], xt[p0:O, 0:l1], xt[p0:O, 257 : 257 + l1]
                )
                nc.vector.tensor_add(
                    s2[p0:O, 0:l2], xt[p0:O, 0:l2], xt[p0:O, 514 : 514 + l2]
                )
                return xt, s1, s2

            def emit_group(wa0, na, wb0, nb, srcs, w0a, w0b, sub):
                # srcs = (xt, s1, s2); na/nb windows for row groups A/B
                pa = ps.tile([O, 1024], F32, tag="pa", name="pa") if na else None
                pb = ps.tile([O, 1024], F32, tag="pb", name="pb") if nb else None
                ntap = len(TAPS)
                for ti, (_, src, off) in enumerate(TAPS):
                    st = srcs[src]
                    lhs_a = wt[0:C, 128 * ti : 128 * ti + 128]
                    lhs_b = wt[C:O, 128 * ti : 128 * ti + 128]
                    first = ti == 0
                    last = ti == ntap - 1
                    for j in range(max(na, nb)):
                        if j < na:
                            o = 512 * (wa0 + j - w0a) + off
                            nc.tensor.matmul(
                                pa[:, 512 * j : 512 * j + 512],
                                lhs_a, st[0:C, o : o + 512],
                                start=first, stop=last,
                            )
                        if j < nb:
                            o = 512 * (wb0 + j - w0b) + off
                            nc.tensor.matmul(
                                pb[:, 512 * j : 512 * j + 512],
                                lhs_b, st[C:O, o : o + 512],
                                start=first, stop=last,
                            )
                ot = outp.tile([O, 2048], BF16, tag="ot", name="ot")
                if na:
                    nc.scalar.copy(ot[:, 0 : 512 * na], pa[:, 0 : 512 * na])
                if nb:
                    if sub % 2 == 0:
                        nc.scalar.copy(
                            ot[:, 1024 : 1024 + 512 * nb], pb[:, 0 : 512 * nb]
                        )
                    else:
                        nc.vector.tensor_copy(
                            ot[:, 1024 : 1024 + 512 * nb], pb[:, 0 : 512 * nb]
                        )
                if na:
                    nc.sync.dma_start(
                        y[:, 512 * wa0 : 512 * (wa0 + na)], ot[:, 0 : 512 * na]
                    )
                if nb:
                    nc.sync.dma_start(
                        y[:, 512 * wb0 : 512 * (wb0 + nb)],
                        ot[:, 1024 : 1024 + 512 * nb],
                    )

            for tblk in range(8):
                w0a = 8 * tblk
                w0b = NW_A + 8 * tblk
                srcs = make_tiles(w0a, w0b, TLEN, L1, L2)
                for sub in range(4):
                    emit_group(
                        w0a + 2 * sub, 2, w0b + 2 * sub, 2, srcs, w0a, w0b, sub
                    )
                if tblk == 0:
                    # final window 128 (B row group), placed early so its
                    # copy+DMA tail hides under later tiles' compute
                    srcs_m = make_tiles(
                        0, 128, TLEN_MINI, TLEN_MINI - 257, TLEN_MINI - 514,
                        b_only=True,
                    )
                    emit_group(0, 0, 128, 1, srcs_m, 0, 128, 1)

    nc.compile()
    return nc


_NC_CACHE = None


def _get_nc():
    global _NC_CACHE
    if _NC_CACHE is None:
        _NC_CACHE = build_kernel()
    return _NC_CACHE


def _host_weights(freq, theta, sigma, psi, f0, theta0, xg, yg):
    """6 unique [64,128] weight blocks, layout [K=128, 6*128] bf16."""
    freq = np.asarray(freq, np.float32)
    theta = np.asarray(theta, np.float32)
    sigma = np.asarray(sigma, np.float32)
    psi = np.asarray(psi, np.float32)
    f0v = float(np.asarray(f0).reshape(-1)[0])
    th0 = float(np.asarray(theta0).reshape(-1)[0])
    xg = np.asarray(xg, np.float32)
    yg = np.asarray(yg, np.float32)
    lsg = 2.0 * np.log(sigma / f0v)
    g_ang = np.exp(-((theta - th0) ** 2) / (2.0 * sigma**2))
    norm = 1.0 / (2.0 * math.pi * sigma**2)
    blocks = []
    for (ky, kx), _, _ in TAPS:
        r = math.sqrt(xg[ky, kx] ** 2 + yg[ky, kx] ** 2 + 0.001)
        g_rad = np.exp(-(((math.log(r) - math.log(f0v)) / lsg) ** 2))
        wb = g_rad * g_ang * np.cos(freq * r + psi) * norm  # [O, I]
        blocks.append(wb.T)  # [I=64, O=128]
    wt = np.concatenate(blocks, axis=1)  # [64, 768]
    return np.concatenate([wt, wt], axis=0).astype(BF16_NP)  # [128, 768]


def kernel(input_tensor, freq, theta, sigma, psi, f0, theta0, xg, yg):
    xb = np.asarray(input_tensor, np.float32).astype(BF16_NP)
    wt = _host_weights(freq, theta, sigma, psi, f0, theta0, xg, yg)
    nc = _get_nc()
    in_maps = []
    for c in range(NB):
        xp = np.zeros((C, XLEN), BF16_NP)
        view = xp[:, GUARD : GUARD + SL].reshape(C, H + 2, WP)
        view[:, 1 : H + 1, 1 : W + 1] = xb[c]
        in_maps.append({"x": xp, "wt": wt})
    res = run_bass_kernel_spmd(nc, in_maps, core_ids=list(range(NB)))
    out = np.empty((NB, O, H, W), np.float32)
    for c in range(NB):
        yv = np.asarray(res.results[c]["y"], dtype=np.float32)
        out[c] = yv.reshape(O, H, WP)[:, :, 1 : W + 1]
    return out
